# revision 26
# baseline (speedup 1.0000x reference)
"""Bass/Trainium2 kernel for elementwise Bessel J2 (nn_BesselFunction).

Input:  x float32 [64, 1048576], values in [0, 30)
Output: J2(x) float32 [64, 1048576]

Sharding: trivially data-parallel; row-block shard across 8 NeuronCores.
Each core sees a [128, 65536] view of its 8-row slice.

Split across engines (DVE is the bottleneck; ACT runs ~1 elem/lane/cycle
with slack). FREE=4096 tiles, bufs=1 intermediates (~180KB SBUF).
  DVE (7 ops/tile): rf=1/x approx, small-branch s*P6(s) in 2 fused passes,
      phase (x - 3pi/4 + g(u)*r via fused custom ops; the -3pi/4 rides
      RED's C3 latch), fused round+range-reduction (magic-number trick),
      final (deg-2 amp polish)*sin multiply, predicated select.
  ACT (3 ops/tile): sin(th) (table, ~9e-8 abs err on [-pi,pi]);
      amp seed sqrt(0.4229*rf - 4.34e-6) ~ Bessel modulus M(x) (composite
      with the fused polish: rel 4.2e-3, scales with sin so harness-rel
      stays bounded); mask = uint8(Sign(6-x)) — the saturating
      float->uint8 cast maps -1 to 0, giving an exact 0/1 mask.
Branch point T=6: small branch (x<6) is s*P(s) deg-6 in s=x^2; big branch
is M(x)*sin(theta).  Dead lanes may hold Inf/NaN; never read.
"""

import os
import sys

import numpy as np

for _p in ("/opt/trn_rl_repo", os.path.expanduser("~/.axon_site/_ro/trn_rl_repo")):
    if os.path.isdir(_p) and _p not in sys.path:
        sys.path.insert(0, _p)

# ---------------------------------------------------------------- constants
T_SPLIT = 6.0
# small branch: J2 ~ s*P(s), s=x^2, P deg-6 on [0, 36]
SM = (0.12499893312003219, -0.010415898038208572, 0.00032533311407436993,
      -5.403510668591466e-06, 5.514329784766668e-08, -3.555001974675806e-10,
      1.1841244857594488e-12)
# big branch phase correction g(u) = G0 + G1*u + G2*u^2 (theta += r*g), [6,30]
G0, G1, G2 = 1.8750133598010976, -0.3556123919235872, -1.1327654353088807
# reciprocal on ACT: rf = Exp(-Ln(x)), rel err 1.22e-5 (phase corr scales it
# by ~corr(6)=0.31 -> ~1.2e-6 abs out err, fine)
# amplitude: ACT power-law seed a = Exp(EP*Ln(x) + EC), then free deg-2
# polish fused into the final multiply: amp = (H2*a + H1)*a + H0 (rel 9.7e-4)
EP, EC = -0.4486626289063497, 0.002433413279171576
H2, H1, H0 = 0.3367337029680267, 0.5907714596744273, 0.0012227529846336653
# DVE fallback amplitude: deg-3 poly in r (rel 1.9e-3)
K0, K1, K2, K3 = (0.06645656499418504, 2.727431071636531,
                  -11.00704359988912, 25.86708690263115)
B34 = 2.3561944901923448       # 3*pi/4
INV_2PI = 0.15915494309189535
MAGIC = 12582912.0             # 1.5 * 2^23
TWO_PI = 6.283185307179586

AMP_ON_ACT = True
# uint8 mask straight out of ACT Sign(T-x) (saturating cast maps -1 -> 0);
# False falls back to DVE is_lt.
MASK_ON_ACT = True

P = 128
COLS = 65536          # per-core elements / 128 partitions
FREE = 4096           # tile free dim
N_CORES = 8

_CACHE: dict = {}


def _register_custom_ops():
    from concourse import dve_ops
    from concourse.dve_spec import (
        Spec, Src0, Src1, C0, C1, C2, C3, sq, lower, _has_src1,
        _spill_c3_to_src1,
    )
    from concourse.dve_uop import DveOpSpec

    def register_op(name, spec):
        for op in dve_ops.OPS:
            if op.name == name:
                return op
        row = max(dve_ops._SUB_OPCODE_FOR_NAME.values()) + 1
        assert row < 0x20, "out of custom-DVE opcode rows"
        dve_ops._SUB_OPCODE_FOR_NAME[name] = row
        shas = {}
        for ver in ("v3", "v4"):
            try:
                s = DveOpSpec(name=name, opcode=row, uops=lower(spec, ver=ver),
                              rd1_en=_has_src1(spec))
                shas[ver] = s.sha(ver)
            except Exception:
                if ver == "v3":
                    raise
        op = dve_ops.DveOp(name, spec, subdim=False, uops_sha=shas)
        dve_ops.OPS.append(op)
        dve_ops.CUSTOM_DVE_SPECS[name] = spec
        return op

    ops = {}
    # w = ((c0*s + c1)*s + c2)*s + c3,  s = in0^2; c3 rides in1 ([P,1] latch)
    s0 = sq(Src0)
    ops["POLY4S"] = register_op("J2_POLY4S", Spec(
        body=_spill_c3_to_src1(((C0 * s0 + C1) * s0 + C2) * s0 + C3),
        reference=lambda in0, in1, c0, c1, c2:
            ((c0 * in0 * in0 + c1) * (in0 * in0) + c2) * (in0 * in0)
            + in1.reshape(-1, 1),
    ))
    # w = (((in0*s + c0)*s + c1)*s + c2) * s,  s = in1^2  (small-branch tail)
    s1c = sq(Src1)
    ops["HORN3S"] = register_op("J2_HORN3S", Spec(
        body=(((Src0 * s1c + C0) * s1c + C1) * s1c + C2) * s1c,
        reference=lambda in0, in1, c0, c1, c2:
            (((in0 * in1 * in1 + c0) * (in1 * in1) + c1) * (in1 * in1) + c2)
            * (in1 * in1),
    ))
    # w = ((c0*v + c1)*v + c2)*v + c3  (amplitude fallback; c3 via in1)
    ops["CUBE4"] = register_op("J2_CUBE4", Spec(
        body=_spill_c3_to_src1(((C0 * Src0 + C1) * Src0 + C2) * Src0 + C3),
        reference=lambda in0, in1, c0, c1, c2:
            ((c0 * in0 + c1) * in0 + c2) * in0 + in1.reshape(-1, 1),
    ))
    # y = in0 + c3; th = y - ((y*c0 + c1) - c1)*c2   (round + range reduce)
    y = Src0 + C3
    ops["RED"] = register_op("J2_RED", Spec(
        body=_spill_c3_to_src1(y - ((y * C0 + C1) - C1) * C2),
        reference=lambda in0, in1, c0, c1, c2: (
            lambda yy: yy - (np.float32(np.float32(yy * np.float32(c0))
                             + np.float32(c1)) - np.float32(c1)) * np.float32(c2)
        )(np.float32(in0 + in1.reshape(-1, 1))),
    ))
    # xp = in1 + ((c0*u + c1)*u + c2)*in0,  u = in0^2  (phase)
    s0p = sq(Src0)
    ops["PHASE"] = register_op("J2_PHASE", Spec(
        body=Src1 + ((C0 * s0p + C1) * s0p + C2) * Src0,
        reference=lambda in0, in1, c0, c1, c2:
            in1 + ((c0 * in0 * in0 + c1) * (in0 * in0) + c2) * in0,
    ))
    # ot = ((c0*a + c1)*a + c2) * sb  (amp polish fused into final multiply)
    ops["AMPFIX"] = register_op("J2_AMPFIX", Spec(
        body=((C0 * Src0 + C1) * Src0 + C2) * Src1,
        reference=lambda in0, in1, c0, c1, c2:
            ((c0 * in0 + c1) * in0 + c2) * in1,
    ))
    return ops


def _build_program(repeat: int = 1, free: int = FREE):
    key = (repeat, free)
    if key in _CACHE:
        return _CACHE[key]

    from contextlib import ExitStack, nullcontext

    import concourse.bacc as bacc
    import concourse.bass as bass
    import concourse.tile as tile
    from concourse import mybir

    ops = _register_custom_ops()
    f32 = mybir.dt.float32
    ALU = mybir.AluOpType
    AF = mybir.ActivationFunctionType
    nt = COLS // free

    nc = bacc.Bacc("TRN2", target_bir_lowering=False, debug=False)
    x_d = nc.dram_tensor("x", [P, COLS], f32, kind="ExternalInput")
    o_d = nc.dram_tensor("out", [P, COLS], f32, kind="ExternalOutput")
    x_ap = x_d.ap()
    o_ap = o_d.ap()

    cd = nc.vector._custom_dve

    with tile.TileContext(nc) as tc, ExitStack() as ctx:
        pools = {}
        # xt/ot double-buffered for DMA overlap; intermediates are produced
        # and consumed in DVE/ACT program order within one tile, so bufs=1
        # is stall-free and keeps FREE=4096 under the SBUF budget (~180KB).
        for name, bufs in (("xt", 2), ("mk", 1), ("rf", 1), ("w1", 1),
                           ("sm", 1), ("xp", 1), ("th", 1), ("sb", 1),
                           ("am", 1), ("ln", 1), ("ot", 2)):
            pools[name] = ctx.enter_context(tc.tile_pool(name=name, bufs=bufs))
        cpool = ctx.enter_context(tc.tile_pool(name="cn", bufs=1))

        def pt(pool, tag=None, dtype=None):
            return pools[pool].tile([P, free], dtype or f32, name=tag or pool,
                                    tag=tag or pool)

        # [P,1] scalar columns: C3 latches + ACT biases
        # 0: SM[3] (small S1 c3)   1: -3pi/4 (RED c3)   2: K0 (CUBE4 c3)
        # 3: T_SPLIT (mask bias)   4: EC (amp-exp bias)  5: 0.0 (sin/rf bias)
        cn = cpool.tile([P, 8], f32, name="cn", tag="cn")
        for i, val in enumerate((SM[3], -B34, K0, T_SPLIT, EC, 0.0)):
            nc.vector.memset(cn[:, i:i + 1], float(np.float32(val)))

        loop_cm = tc.For_i(0, repeat, 1) if repeat > 1 else nullcontext()
        with loop_cm:
          for i in range(nt):
            sl = bass.ts(i, free)
            xt = pt("xt")
            nc.sync.dma_start(xt[:], x_ap[:, sl])

            # --- ACT head: ln -> rf=exp(-ln), amp-seed=exp(EP*ln+EC), mask.
            # All in the natural_log_exp table set (Sign is filler) so one
            # table switch; DVE fills the latency with phase + small branch.
            ln = pt("ln")
            nc.scalar.activation(ln[:], xt[:], AF.Ln)
            rf = pt("rf")
            nc.scalar.activation(rf[:], ln[:], AF.Exp, scale=-1.0,
                                 bias=cn[:, 5:6])
            am = pt("am")
            nc.scalar.activation(am[:], ln[:], AF.Exp, scale=EP,
                                 bias=cn[:, 4:5])

            xp = pt("xp")
            cd(ops["PHASE"], out=xp[:], in0=rf[:], in1=xt[:],
               s0=G2, s1=G1, imm2=G0)
            th = pt("th")
            cd(ops["RED"], out=th[:], in0=xp[:], in1=cn[:, 1:2],
               s0=INV_2PI, s1=MAGIC, imm2=TWO_PI)
            sb = pt("sb")
            nc.scalar.activation(sb[:], th[:], AF.Sin, bias=cn[:, 5:6])
            mk = pt("mk", dtype=mybir.dt.uint8)
            # uint8(Sign(T-x)): 1 iff x < T (saturating cast: -1 -> 0);
            # placed after Sin so it rides whatever set is loaded (filler fn)
            nc.scalar.activation(mk[:], xt[:], AF.Sign, scale=-1.0,
                                 bias=cn[:, 3:4])
            w1 = pt("w1")
            cd(ops["POLY4S"], out=w1[:], in0=xt[:], in1=cn[:, 0:1],
               s0=SM[6], s1=SM[5], imm2=SM[4])
            sm = pt("sm")
            cd(ops["HORN3S"], out=sm[:], in0=w1[:], in1=xt[:],
               s0=SM[2], s1=SM[1], imm2=SM[0])

            ot = pt("ot")
            if AMP_ON_ACT:
                cd(ops["AMPFIX"], out=ot[:], in0=am[:], in1=sb[:],
                   s0=H2, s1=H1, imm2=H0)
            else:
                cd(ops["CUBE4"], out=am[:], in0=rf[:], in1=cn[:, 2:3],
                   s0=K3, s1=K2, imm2=K1)
                nc.vector.tensor_tensor(ot[:], am[:], sb[:], ALU.mult)

            nc.vector.copy_predicated(ot[:], mk[:], sm[:])
            nc.sync.dma_start(o_ap[:, sl], ot[:])

    nc.compile()
    _CACHE[key] = {"nc": nc}
    return _CACHE[key]


def kernel(x: np.ndarray) -> np.ndarray:
    from concourse import bass_utils

    prog = _build_program()
    x = np.asarray(x, dtype=np.float32)
    rows = x.shape[0] // N_CORES
    in_maps = [
        {"x": np.ascontiguousarray(
            x[rows * k: rows * (k + 1)].reshape(P, COLS))}
        for k in range(N_CORES)
    ]
    res = bass_utils.run_bass_kernel_spmd(
        prog["nc"], in_maps, core_ids=list(range(N_CORES)))
    out = np.concatenate(
        [res.results[k]["out"].reshape(rows, -1) for k in range(N_CORES)], axis=0)
    return out.astype(np.float32)
